# revision 1
# baseline (speedup 1.0000x reference)
"""Fused graph Fokker-Planck ODE function kernel for Trainium2 (8 NeuronCores).

Sharding: data-parallel over batch B=4 x row-halves (i in [0,256) / [256,512))
-> 8 shards.  Each core computes dh_dt for one (batch, i-half) pair.

Math (per batch; all [i,j] matrices kept transposed as [j,i] on chip so the
j-contraction matmuls need no transposes):
    S      = A * (K @ Q^T) / sqrt(D)          (elementwise mask, no -inf)
    X      = exp(S)                            (unnormalized softmax; masked
                                                scores are O(5) so no max sub)
    sg     = sigmoid(10*(E_j - E_i)) = 1 - 1/(1+exp(10*(E_j-E_i)))
    M4     = X * (1-sg)                        (M3 = X*sg is never formed:
                                                G3 = X^T@B - G4)
    G3     = M3^T @ [h | E*h | L*h | 1]       (L = log(h+1e-8))
    G4     = M4^T @ [E | L | 1]
    s_i    = r3 + r4                           (softmax denominator)
    dh[i,d] = (1/s_i) * ( G3Eh - E_i*G3h + h_i*(G4E - E_i*r4)
                        + beta_d*( G3Lh - L_i*(G3h + r4*h_i) + h_i*G4L ) )
"""

import math
import os as _os
import sys

import numpy as np

for _p in ("/opt/trn_rl_repo",):
    if _p not in sys.path:
        sys.path.insert(0, _p)

B, N, D, PED = 4, 512, 32, 16
NCORES = 8
RPC = N // 2            # i-rows per core
NJT = N // 128          # j tiles of 128
NIT = RPC // 128        # i tiles of 128
BLK = 256               # rhs block stride (padded for 1-cycle/row matmuls)
GW = 131                # used columns per G-result block
KSH = 10.0
ISD = 1.0 / math.sqrt(D)

# float32r (single-pass matmuls, pre-rounded producer tiles)
R_S = _os.environ.get("KR_S", "1") == "1"
R_ACC = _os.environ.get("KR_ACC", "1") == "1"
R_QK = _os.environ.get("KR_QK", "1") == "1"

_CACHE = {}


def _patch_act_tables():
    """Make natural_log_exp_and_others the only ACT table set containing our
    functions (exp/ln/identity/copy) so bacc emits exactly one
    ACT_TABLE_LOAD.  Dict length/order is preserved — the set INDEX is the
    runtime act_func_set_id, so entries must not be removed."""
    import concourse.bacc as bacc_mod
    if getattr(bacc_mod, "_act_tables_patched", False):
        return
    orig = bacc_mod.get_activation_tables

    def filtered(arch):
        t = orig(arch)
        target = t.get("natural_log_exp_and_others")
        if not target:
            return t
        return {k: (v if k == "natural_log_exp_and_others" else (v - target))
                for k, v in t.items()}

    bacc_mod.get_activation_tables = filtered
    bacc_mod._act_tables_patched = True


def _build_program():
    import concourse.bacc as bacc
    import concourse.tile as tile
    from concourse import mybir
    from contextlib import ExitStack

    _patch_act_tables()

    fp32 = mybir.dt.float32
    f32r = mybir.dt.float32r
    AF = mybir.ActivationFunctionType
    ADD, MUL = mybir.AluOpType.add, mybir.AluOpType.mult

    dtS = f32r if R_S else fp32
    dtA = f32r if R_ACC else fp32
    dtQ = f32r if R_QK else fp32

    nc = bacc.Bacc("TRN2", target_bir_lowering=False, debug=False,
                   num_devices=NCORES)

    def din(name, shape):
        return nc.dram_tensor(name, shape, fp32, kind="ExternalInput").ap()

    AT = din("AT", [128, NJT * RPC])     # host-permuted [p, (t i)]
    hj = din("hj", [128, NJT * D])       # host-permuted [p, (t d)]
    hi = din("hi", [128, NIT * D])       # host-permuted [p, (t d)]
    pe2 = din("pe2", [PED, N + RPC])      # [peT | peiT]
    wkq = din("wkq", [PED, 2 * D])        # [Wk | Wq]
    smalls = din("smalls", [128, 8])      # [Ej(4) | Ei(2) | bk | bq]
    rows1 = din("rows1", [1, RPC + D])    # [Ei row | beta]
    out = nc.dram_tensor("out", [128, NIT * D], fp32,
                         kind="ExternalOutput").ap()

    with tile.TileContext(nc) as tc, ExitStack() as ctx:
        cst = ctx.enter_context(tc.tile_pool(name="cst", bufs=1))
        sb = ctx.enter_context(tc.tile_pool(name="sb", bufs=1))
        keep = ctx.enter_context(tc.tile_pool(name="keep", bufs=1))
        fin = ctx.enter_context(tc.tile_pool(name="fin", bufs=1))
        pq = ctx.enter_context(tc.tile_pool(name="pq", bufs=1, space="PSUM"))
        sps = ctx.enter_context(tc.tile_pool(name="sps", bufs=1, space="PSUM"))
        fps = ctx.enter_context(tc.tile_pool(name="fps", bufs=NIT, space="PSUM"))

        # ------------- input loads (A via gpsimd queue, rest via sync) -----
        rows1_sb = cst.tile([1, RPC + D], fp32, tag="rows1_sb")
        nc.scalar.dma_start(rows1_sb[:], rows1[:])
        wkq_sb = cst.tile([PED, 2 * D], fp32, tag="wkq_sb")
        nc.scalar.dma_start(wkq_sb[:], wkq[:])
        pe_sb = cst.tile([PED, N + RPC], fp32, tag="pe_sb")
        nc.sync.dma_start(pe_sb[:, 0:N // 2], pe2[:, 0:N // 2])
        nc.scalar.dma_start(pe_sb[:, N // 2:N], pe2[:, N // 2:N])
        nc.sync.dma_start(pe_sb[:, N:N + RPC], pe2[:, N:N + RPC])
        smalls_sb = cst.tile([128, 8], fp32, tag="smalls_sb")
        nc.sync.dma_start(smalls_sb[:], smalls[:])
        hj_sb = cst.tile([128, NJT * D], fp32, tag="hj_sb")
        hv = hj_sb.rearrange("p (t d) -> p t d", d=D)
        nc.scalar.dma_start(hj_sb[:], hj[:])
        hi_all = fin.tile([128, NIT * D], fp32, tag="hi_all")
        hiv = hi_all.rearrange("p (t d) -> p t d", d=D)
        nc.scalar.dma_start(hi_all[:], hi[:])
        at_all = cst.tile([128, NJT * RPC], fp32, tag="at_all")
        HW0 = NJT * RPC // 2
        nc.sync.dma_start(at_all[:, 0:HW0], AT[:, 0:HW0])
        nc.scalar.dma_start(at_all[:, HW0:2 * HW0], AT[:, HW0:2 * HW0])

        peT_sb, peiT_sb = pe_sb[:, 0:N], pe_sb[:, N:N + RPC]
        wk_sb, wq_sb = wkq_sb[:, 0:D], wkq_sb[:, D:2 * D]
        ej_sb = smalls_sb[:, 0:NJT]
        ei_sb = smalls_sb[:, NJT:NJT + NIT]
        bk_sb = smalls_sb[0:D, 6:7]
        bq_sb = smalls_sb[0:D, 7:8]
        eirow_sb = rows1_sb[:, 0:RPC]
        betarow_sb = rows1_sb[:, RPC:RPC + D]

        zero1 = cst.tile([128, 1], fp32, tag="zero1")
        nc.vector.memset(zero1[:], 0.0)
        eps1 = cst.tile([128, 1], fp32, tag="eps1")
        nc.vector.memset(eps1[:], 1e-8)
        # dummy first ACT op: hoists the one ACT_TABLE_LOAD off the
        # critical path (it otherwise waits for the first real input)
        warm = cst.tile([128, 1], fp32, tag="warm")
        nc.scalar.activation(warm[:], zero1[:], AF.Exp, bias=zero1[:])
        if R_QK:
            wkq_r = cst.tile([PED, 2 * D], dtQ, tag="wkq_r")
            nc.vector.tensor_copy(wkq_r[:], wkq_sb[:])
            pe_r = cst.tile([PED, N + RPC], dtQ, tag="pe_r")
            nc.vector.tensor_copy(pe_r[:, 0:N], pe_sb[:, 0:N])
            nc.vector.tensor_copy(pe_r[:, N:N + RPC], pe_sb[:, N:N + RPC])
            peT_sb, peiT_sb = pe_r[:, 0:N], pe_r[:, N:N + RPC]
            wk_sb, wq_sb = wkq_r[:, 0:D], wkq_r[:, D:2 * D]
        e10_sb = cst.tile([128, NJT], fp32, tag="e10")    # 10*E_j
        nc.vector.tensor_scalar_mul(e10_sb[:], ej_sb, KSH)

        # ------------- E_i broadcast early (PE is idle here) -------------
        # f32r rank-1: single-pass, 1 cycle/row (fp32 would be 2-pass 4c/row
        # and delay Q/K behind it on the PE stream)
        ones1 = cst.tile([1, 128], dtQ, tag="ones1")
        nc.vector.memset(ones1.bitcast(fp32)[:], 1.0)
        eirow_r = cst.tile([1, RPC], dtQ, tag="eirow_r")
        nc.vector.tensor_copy(eirow_r[:], eirow_sb)
        bcps = pq.tile([128, RPC], fp32, tag="qk2")
        nc.tensor.matmul(bcps[:], ones1[:], eirow_r[:], start=True, stop=True)
        eib_sb = cst.tile([128, RPC], fp32, tag="eib")    # E_i bcast
        nc.vector.tensor_copy(eib_sb[:], bcps[:])

        # ------------- K^T, Q^T -------------
        qps = pq.tile([D, N], fp32, tag="qk")
        nc.tensor.matmul(qps[:], wq_sb, peT_sb, start=True, stop=True)
        qT_sb = cst.tile([D, N], dtS, tag="qT")
        nc.scalar.activation(qT_sb[:], qps[:], AF.Identity, bias=bq_sb,
                             scale=1.0)

        kps = pq.tile([D, RPC], fp32, tag="qk2")
        nc.tensor.matmul(kps[:], wk_sb, peiT_sb, start=True, stop=True)
        kT_sb = cst.tile([D, RPC], dtS, tag="kT")
        # (K + bk) * (1/sqrt(D))
        nc.vector.tensor_scalar(kT_sb[:], kps[:], bk_sb, ISD, op0=ADD, op1=MUL)

        # ------------- rhs blocks [h | E*h | L*h | 1 | E | L | 1 | pad] ----
        rhs_all = keep.tile([128, NJT * BLK], dtA, tag="rhs_all")
        rv = rhs_all.rearrange("p (t c) -> p t c", c=BLK)
        nc.vector.tensor_copy(rv[:, :, 0:D], hv[:])
        nc.scalar.activation(rv[:, :, 98:130], hv[:], AF.Ln, bias=eps1[:])  # L
        nc.vector.tensor_mul(rv[:, :, 2 * D:3 * D], rv[:, :, 98:130], hv[:])
        for jt in range(NJT):
            nc.vector.tensor_scalar_mul(
                rhs_all[:, jt * BLK + D:jt * BLK + 2 * D],
                hj_sb[:, jt * D:(jt + 1) * D], ej_sb[:, jt:jt + 1])  # E*h
        nc.vector.tensor_copy(rv[:, :, 97:98],
                              ej_sb.rearrange("p (t o) -> p t o", o=1))  # E
        onesjt = cst.tile([128, NJT], fp32, tag="onesjt")
        nc.vector.memset(onesjt[:], 1.0)
        ojv = onesjt.rearrange("p (t o) -> p t o", o=1)
        nc.vector.tensor_copy(rv[:, :, 96:97], ojv[:])
        nc.vector.tensor_copy(rv[:, :, 130:131], ojv[:])
        # cols 131:BLK are read by the padded matmuls but their output
        # columns are never consumed; zero them (same bit pattern in f32r)
        # so nothing reads uninitialized memory.
        nc.vector.memset(rv[:, :, GW:BLK].bitcast(fp32), 0.0)

        # ---- scores + masked exp + sign split, pipelined in two halves ----
        HW2 = 2 * RPC            # half width (2 j-tiles)
        sall = sps.tile([128, NJT * RPC], fp32, tag="sall")
        X = keep.tile([128, NJT * RPC], dtA, tag="X")
        M4 = keep.tile([128, NJT * RPC], dtA, tag="M4")
        ez = sb.tile([128, NJT * RPC], fp32, tag="ez")
        d1 = sb.tile([128, NJT * RPC], fp32, tag="d1")
        rd = sb.tile([128, NJT * RPC], fp32, tag="rd")    # 1-sg, ~18 bits
        msk = sb.tile([128, NJT * RPC], fp32, tag="msk")
        for hh in range(2):
            h0 = hh * HW2
            sl = slice(h0, h0 + HW2)
            for jt in (2 * hh, 2 * hh + 1):
                nc.tensor.matmul(sall[:, jt * RPC:(jt + 1) * RPC],
                                 qT_sb[:, jt * 128:(jt + 1) * 128],
                                 kT_sb[:], start=True, stop=True)
                nc.scalar.activation(ez[:, jt * RPC:(jt + 1) * RPC], eib_sb[:],
                                     AF.Exp, bias=e10_sb[:, jt:jt + 1],
                                     scale=-KSH)
            nc.vector.tensor_scalar_add(d1[:, sl], ez[:, sl], 1.0)
            nc.vector.reciprocal_approx_fast(rd[:, sl], d1[:, sl])
            nc.vector.tensor_mul(msk[:, sl], at_all[:, sl], sall[:, sl])
            nc.scalar.activation(X[:, sl], msk[:, sl], AF.Exp, bias=zero1[:])
            nc.vector.tensor_mul(M4[:, sl], X[:, sl], rd[:, sl])

        # ------------- beta broadcast (needed only in finals) -------------
        betarow_r = cst.tile([1, D], dtQ, tag="betarow_r")
        nc.vector.tensor_copy(betarow_r[:], betarow_sb)
        bcps2 = pq.tile([128, D], fp32, tag="qk2")
        nc.tensor.matmul(bcps2[:], ones1[:], betarow_r[:], start=True, stop=True)
        betab_sb = cst.tile([128, D], fp32, tag="betab")  # beta bcast
        nc.vector.tensor_copy(betab_sb[:], bcps2[:])

        # ------------- accumulation matmuls (lhsT in {X, M4}) -------------
        SUB = mybir.AluOpType.subtract
        g_all = fin.tile([128, NIT * GW], fp32, tag="g_all")
        for it in range(NIT):
            i0 = it * 128
            ppx = fps.tile([128, BLK], fp32, tag="ppx")
            pp4 = fps.tile([128, BLK], fp32, tag="pp4")
            for jt in range(NJT):
                st, sp = (jt == 0), (jt == NJT - 1)
                blk = rhs_all[:, jt * BLK:(jt + 1) * BLK]
                nc.tensor.matmul(ppx[:], X[:, jt * RPC + i0:jt * RPC + i0 + 128],
                                 blk, start=st, stop=sp)
                nc.tensor.matmul(pp4[:], M4[:, jt * RPC + i0:jt * RPC + i0 + 128],
                                 blk, start=st, stop=sp)
            # g block = [G3h G3Eh G3Lh r3 | G4E G4L r4];  G3 = GX - G4
            g4h = fin.tile([128, 97], fp32, tag="g4h")
            nc.vector.tensor_copy(g4h[:], pp4[:, 0:97])
            nc.vector.tensor_copy(g_all[:, it * GW + 97:(it + 1) * GW],
                                  pp4[:, 97:GW])
            nc.vector.tensor_tensor(g_all[:, it * GW:it * GW + 97],
                                    ppx[:, 0:97], g4h[:], op=SUB)

        # ------------- finals, consolidated over both i-tiles -------------
        gvw = g_all.rearrange("p (t c) -> p t c", c=GW)
        G3h, G3Eh, G3Lh = gvw[:, :, 0:D], gvw[:, :, D:2 * D], gvw[:, :, 2 * D:3 * D]
        r3, G4E = gvw[:, :, 96:97], gvw[:, :, 97:98]
        G4L, r4 = gvw[:, :, 98:130], gvw[:, :, 130:131]
        eivw = ei_sb.rearrange("p (t o) -> p t o", o=1)

        def bc(ap):  # [128, NIT, 1] -> broadcast along d
            return ap.to_broadcast((128, NIT, D))

        li_all = fin.tile([128, NIT, D], fp32, tag="li_all")
        nc.scalar.activation(li_all[:], hiv[:], AF.Ln, bias=eps1[:])

        s_all = fin.tile([128, NIT], fp32, tag="s_all")
        svw = s_all.rearrange("p (t o) -> p t o", o=1)
        nc.vector.tensor_tensor(svw[:], r3, r4, op=ADD)
        invs = fin.tile([128, NIT], fp32, tag="invs")
        nc.vector.reciprocal(invs[:], s_all[:])

        m1 = fin.tile([128, NIT], fp32, tag="m1")
        m1v = m1.rearrange("p (t o) -> p t o", o=1)
        nc.vector.tensor_tensor(m1v[:], eivw[:], r4, op=MUL)
        u_all = fin.tile([128, NIT], fp32, tag="u_all")
        uv = u_all.rearrange("p (t o) -> p t o", o=1)
        nc.vector.tensor_sub(uv[:], G4E, m1v[:])

        v1 = fin.tile([128, NIT, D], fp32, tag="v1")
        nc.vector.tensor_mul(v1[:], bc(eivw), G3h)
        t1_all = fin.tile([128, NIT, D], fp32, tag="t1_all")
        nc.vector.tensor_sub(t1_all[:], G3Eh, v1[:])
        v2 = fin.tile([128, NIT, D], fp32, tag="v2")
        nc.vector.tensor_mul(v2[:], hiv[:], bc(uv))
        t12_all = fin.tile([128, NIT, D], fp32, tag="t12_all")
        nc.vector.tensor_add(t12_all[:], t1_all[:], v2[:])

        v3 = fin.tile([128, NIT, D], fp32, tag="v3")
        nc.vector.tensor_mul(v3[:], hiv[:], bc(r4))
        w_all = fin.tile([128, NIT, D], fp32, tag="w_all")
        nc.vector.tensor_add(w_all[:], G3h, v3[:])
        z_all = fin.tile([128, NIT, D], fp32, tag="z_all")
        nc.vector.tensor_mul(z_all[:], li_all[:], w_all[:])
        e1_all = fin.tile([128, NIT, D], fp32, tag="e1_all")
        nc.vector.tensor_sub(e1_all[:], G3Lh, z_all[:])
        q_all = fin.tile([128, NIT, D], fp32, tag="q_all")
        nc.vector.tensor_mul(q_all[:], hiv[:], G4L)
        e2_all = fin.tile([128, NIT, D], fp32, tag="e2_all")
        nc.vector.tensor_add(e2_all[:], e1_all[:], q_all[:])
        bt_all = fin.tile([128, NIT, D], fp32, tag="bt_all")
        bvw = betab_sb.rearrange("p (t d) -> p t d", t=1).to_broadcast((128, NIT, D))
        nc.vector.tensor_mul(bt_all[:], e2_all[:], bvw)
        pre_all = fin.tile([128, NIT, D], fp32, tag="pre_all")
        nc.vector.tensor_add(pre_all[:], t12_all[:], bt_all[:])
        res_all = fin.tile([128, NIT, D], fp32, tag="res_all")
        iv = invs.rearrange("p (t o) -> p t o", o=1)
        nc.vector.tensor_mul(res_all[:], pre_all[:], iv.to_broadcast((128, NIT, D)))
        nc.sync.dma_start(out[:], res_all.rearrange("p t d -> p (t d)"))

    nc.compile()
    return nc


def _get_program():
    if "nc" not in _CACHE:
        _CACHE["nc"] = _build_program()
    return _CACHE["nc"]


def make_in_maps(h, pe, E, A, Wk, bk, Wq, bq, beta):
    f = lambda x: np.ascontiguousarray(np.asarray(x, dtype=np.float32))
    h, pe, E, A = f(h), f(pe), f(E), f(A)
    Wk, bk, Wq, bq, beta = f(Wk), f(bk), f(Wq), f(bq), f(beta)
    wkq = np.concatenate([Wk, Wq], axis=1)
    in_maps = []
    for c in range(NCORES):
        b, r = c // 2, c % 2
        isl = slice(r * RPC, (r + 1) * RPC)
        smalls = np.zeros((128, 8), np.float32)
        smalls[:, 0:NJT] = E.reshape(NJT, 128).T
        smalls[:, NJT:NJT + NIT] = E[isl].reshape(NIT, 128).T
        smalls[0:D, 6] = bk
        smalls[0:D, 7] = bq
        rows1 = np.concatenate([E[isl], beta]).reshape(1, RPC + D)
        atp = A[isl].T.reshape(NJT, 128, RPC).transpose(1, 0, 2)
        hjp = h[b].reshape(NJT, 128, D).transpose(1, 0, 2)
        hip = h[b, isl].reshape(NIT, 128, D).transpose(1, 0, 2)
        in_maps.append({
            "AT": f(atp.reshape(128, NJT * RPC)),
            "hj": f(hjp.reshape(128, NJT * D)),
            "hi": f(hip.reshape(128, NIT * D)),
            "pe2": f(np.concatenate([pe[b].T, pe[b, isl].T], axis=1)),
            "wkq": f(wkq),
            "smalls": smalls,
            "rows1": f(rows1),
        })
    return in_maps


def gather(results):
    out = np.empty((B, N, D), np.float32)
    for c in range(NCORES):
        b, r = c // 2, c % 2
        o = results[c]["out"].reshape(128, NIT, D).transpose(1, 0, 2)
        out[b, r * RPC:(r + 1) * RPC] = o.reshape(RPC, D)
    return out


def _axon_reset():
    try:
        import ctypes
        import jax
        lib = ctypes.CDLL("/opt/axon/libaxon_pjrt.so")
        lib.axon_reset.restype = ctypes.c_int64
        jax.devices()
        lib.axon_reset()
    except Exception:
        pass


def kernel(t=None, h=None, pe=None, E=None, A=None, Wk=None, bk=None,
           Wq=None, bq=None, beta=None, **_unused):
    from concourse.bass_utils import run_bass_kernel_spmd
    nc = _get_program()
    in_maps = make_in_maps(h, pe, E, A, Wk, bk, Wq, bq, beta)
    try:
        res = run_bass_kernel_spmd(nc, in_maps, list(range(NCORES)))
    except Exception:
        # a previously wedged NeuronCore shows up as an opaque runtime
        # error on the first execute — reset the device once and retry
        _axon_reset()
        import time as _time
        _time.sleep(2)
        res = run_bass_kernel_spmd(nc, in_maps, list(range(NCORES)))
    return gather(res.results)



# revision 15
# speedup vs baseline: 1.0653x; 1.0653x over previous
"""Fused graph Fokker-Planck ODE function kernel for Trainium2 (8 NeuronCores).

Sharding: data-parallel over batch B=4 x row-halves (i in [0,256) / [256,512))
-> 8 shards.  Each core computes dh_dt for one (batch, i-half) pair.

Math (per batch; [i,j] matrices kept transposed as [j,i] on chip so the
j-contraction matmuls need no transposes):
    S      = A * (K' @ Q^T)            K' = (pe Wk + bk)/sqrt(D) (host-folded)
    X      = exp(S)                    (unnormalized softmax; masked scores
                                        are O(5) so no max subtraction)
    rd     = sigmoid(10*(E_i - E_j)) = 0.5*tanh(5*(E_i - E_j)) + 0.5
    M4     = X * rd
    GX     = X^T  @ [h | EhL | 1]           (65 cols;  EhL = E*h + beta*L*h)
    G4     = M4^T @ [h | EhL | 1 | EL']     (97 cols;  EL' = E + beta*L)
    G3h    = GXh - G4h;  A = GX_EhL - G4_EhL;  s = sum_j X;  r4 = sum_j M4
    P      = G3h + r4*h_i
    dh     = (A - (E_i + beta*L_i) . P + h_i * G4_EL') / s
All beta factors are folded into the host-packed rhs columns; L = log(h+1e-8)
and the tiny O(N*PE*D) Q/K projections are computed on host.  bf16 on the
N^2 device path (fp32 PSUM accum); measured rel err ~8e-4 vs fp32 reference.
"""

import math
import sys

import numpy as np

for _p in ("/opt/trn_rl_repo",):
    if _p not in sys.path:
        sys.path.insert(0, _p)

import ml_dtypes

B, N, D, PED = 4, 512, 32, 16
NCORES = 8
RPC = N // 2            # i-rows per core
NJT = N // 128          # j tiles of 128
NIT = RPC // 128        # i tiles of 128
GW = 97                 # packed rhs cols per j-tile: [h|EhL|1|EL']
XW = 65                 # X-matmul reads [h|EhL|1]
ISD = 1.0 / math.sqrt(D)
BF16 = ml_dtypes.bfloat16

_CACHE = {}


def _patch_act_tables():
    """Make exp_and_others (exp + tanh + identity) the only ACT table set
    containing our functions so bacc emits exactly one ACT_TABLE_LOAD.
    Dict length/order preserved — the set INDEX is the runtime
    act_func_set_id."""
    import concourse.bacc as bacc_mod
    if getattr(bacc_mod, "_act_tables_patched", False):
        return
    orig = bacc_mod.get_activation_tables

    def filtered(arch):
        t = orig(arch)
        target = t.get("exp_and_others")
        if not target:
            return t
        return {k: (v if k == "exp_and_others" else (v - target))
                for k, v in t.items()}

    bacc_mod.get_activation_tables = filtered
    bacc_mod._act_tables_patched = True


def _build_program():
    import concourse.bacc as bacc
    import concourse.tile as tile
    from concourse import mybir
    from contextlib import ExitStack

    _patch_act_tables()

    fp32 = mybir.dt.float32
    f32r = mybir.dt.float32r
    bf16 = mybir.dt.bfloat16
    AF = mybir.ActivationFunctionType
    ADD, MUL, SUB = (mybir.AluOpType.add, mybir.AluOpType.mult,
                     mybir.AluOpType.subtract)

    nc = bacc.Bacc("TRN2", target_bir_lowering=False, debug=False,
                   num_devices=NCORES)

    def din(name, shape, dt=fp32):
        return nc.dram_tensor(name, shape, dt, kind="ExternalInput").ap()

    erow = din("erow", [1, RPC])               # E_i row (fp32 for tanh)
    qkT = din("qkT", [D, N + RPC], bf16)       # [Q^T | K'^T], host-projected
    at = din("at", [128, NJT * RPC], bf16)     # A[isl].T as [p,(t i)]
    rhsp = din("rhsp", [128, NJT * GW], bf16)  # packed rhs per j-tile
    smalls = din("smalls", [128, 8])           # [-5Ej(4) | -Ei(2) | 0 | 0]
    hili = din("hili", [128, 4 * D])           # [hi (NIT D) | beta*li]
    out = nc.dram_tensor("out", [128, NIT * D], fp32,
                         kind="ExternalOutput").ap()

    with tile.TileContext(nc) as tc, ExitStack() as ctx:
        cst = ctx.enter_context(tc.tile_pool(name="cst", bufs=1))
        fin = ctx.enter_context(tc.tile_pool(name="fin", bufs=1))
        pp1 = ctx.enter_context(tc.tile_pool(name="pp1", bufs=1, space="PSUM"))
        pps = ctx.enter_context(tc.tile_pool(name="pps", bufs=2, space="PSUM"))

        # ---------------- input DMAs, one queue each, critical first -------
        erow_sb = cst.tile([1, RPC], fp32, tag="erow")
        nc.sync.dma_start(erow_sb[:], erow[:])
        qkT_sb = cst.tile([D, N + RPC], bf16, tag="qkT")
        nc.sync.dma_start(qkT_sb[:], qkT[:])
        rhsp_sb = cst.tile([128, NJT * GW], bf16, tag="rhsp")
        nc.sync.dma_start(rhsp_sb[:], rhsp[:])
        smalls_sb = cst.tile([128, 8], fp32, tag="smalls")
        nc.gpsimd.dma_start(smalls_sb[:], smalls[:])
        at_sb = cst.tile([128, NJT * RPC], bf16, tag="at_sb")
        nc.gpsimd.dma_start(at_sb[:], at[:])

        # V: constants (warm-act + PE-warm deps)
        zero1 = cst.tile([128, 1], fp32, tag="zero1")
        nc.vector.memset(zero1[:], 0.0)
        zbf = cst.tile([1, 640], bf16, tag="zbf")
        nc.vector.memset(zbf[:], 0.0)
        ones1 = cst.tile([1, 128], f32r, tag="ones1")
        nc.vector.memset(ones1.bitcast(fp32)[:], 1.0)

        # hoist the single ACT_TABLE_LOAD off the critical path, then the
        # finals-only hili load on the scalar queue
        warm = cst.tile([128, 1], fp32, tag="warm")
        nc.scalar.activation(warm[:], zero1[:], AF.Exp, bias=zero1[:])
        hili_sb = cst.tile([128, 4 * D], fp32, tag="hili")
        nc.scalar.dma_start(hili_sb[:], hili[:])

        # ---------------- PE warm-up (p-state ramp starts ~3.4us early) ----
        qdum = pp1.tile([128, 512], fp32, tag="qdum")
        for _ in range(3):
            nc.tensor.matmul(qdum[:], zbf[:, 0:128], zbf[:, 128:640],
                             start=True, stop=True)

        m5ej = smalls_sb[:, 0:NJT]            # -5*E_j tiles
        negEi = smalls_sb[:, NJT:NJT + NIT]   # -E_i tiles
        hi_all = hili_sb[:, 0:NIT * D]        # h_i, both i-tiles
        li_all = hili_sb[:, NIT * D:2 * NIT * D]  # beta*L_i
        qT = qkT_sb[:, 0:N]
        kT = qkT_sb[:, N:N + RPC]

        # ---------------- E_i broadcast (rank-1 f32r matmul) ---------------
        eirow_r = cst.tile([1, RPC], f32r, tag="eirow_r")
        nc.vector.tensor_copy(eirow_r[:], erow_sb[:])
        ek = pp1.tile([128, RPC], fp32, tag="ek")
        nc.tensor.matmul(ek[:], ones1[:], eirow_r[:], start=True, stop=True)
        eibps = ek[:]

        # ---------------- scores (PE), tanh (S), rd (Pool) -----------------
        salls = []
        for t in range(NJT):
            sps = pps.tile([128, RPC], fp32, tag="sall")
            nc.tensor.matmul(sps[:], qT[:, t * 128:(t + 1) * 128], kT,
                             start=True, stop=True)
            salls.append(sps)

        tanh_sb = cst.tile([128, NJT * RPC], fp32, tag="tanh")
        rd_sb = cst.tile([128, NJT * RPC], bf16, tag="rd")
        msk_sb = cst.tile([128, NJT * RPC], fp32, tag="msk")
        X_sb = cst.tile([128, NJT * RPC], bf16, tag="X")
        M4_sb = cst.tile([128, NJT * RPC], bf16, tag="M4")

        for t in range(NJT):
            sl = slice(t * RPC, (t + 1) * RPC)
            # S: tanh(5*E_i - 5*E_j)
            nc.scalar.activation(tanh_sb[:, sl], eibps, AF.Tanh,
                                 bias=m5ej[:, t:t + 1], scale=5.0)
            # Pool: rd = 0.5*tanh + 0.5  -> bf16
            nc.gpsimd.tensor_scalar(rd_sb[:, sl], tanh_sb[:, sl], 0.5, 0.5,
                                    op0=MUL, op1=ADD)
        for t in range(NJT):
            sl = slice(t * RPC, (t + 1) * RPC)
            nc.vector.tensor_tensor(msk_sb[:, sl], at_sb[:, sl], salls[t][:],
                                    op=MUL)
            nc.scalar.activation(X_sb[:, sl], msk_sb[:, sl], AF.Exp,
                                 bias=zero1[:])
            nc.vector.tensor_tensor(M4_sb[:, sl], X_sb[:, sl], rd_sb[:, sl],
                                    op=MUL)

        # ---------------- accumulation matmuls ----------------------------
        ppxt = [pp1.tile([128, XW], fp32, tag=f"ppx{it}", name=f"ppx{it}")
                for it in range(NIT)]
        pp4t = [pp1.tile([128, GW], fp32, tag=f"pp4{it}", name=f"pp4{it}")
                for it in range(NIT)]
        ppx = [a[:] for a in ppxt]
        pp4 = [a[:] for a in pp4t]
        for t in range(NJT):
            st, sp = (t == 0), (t == NJT - 1)
            rs = rhsp_sb[:, t * GW:t * GW + GW]
            for it in range(NIT):
                xsl = X_sb[:, t * RPC + it * 128:t * RPC + (it + 1) * 128]
                msl = M4_sb[:, t * RPC + it * 128:t * RPC + (it + 1) * 128]
                nc.tensor.matmul(ppx[it], xsl, rs[:, 0:XW], start=st, stop=sp)
                nc.tensor.matmul(pp4[it], msl, rs, start=st, stop=sp)

        # ---------------- finals -------------------------------------------
        # dh*s = A - (E_i + beta*L_i)*P + h_i*G4EL'
        #   with P = G3h + r4*h_i, A = GX_EhL - G4_EhL, G3h = GXh - G4h
        g4sb = fin.tile([128, NIT * XW], fp32, tag="g4sb")
        ssb = fin.tile([128, NIT], fp32, tag="ssb")
        g3h = fin.tile([128, NIT * D], fp32, tag="g3h")
        aa = fin.tile([128, NIT * D], fp32, tag="aa")
        c1 = fin.tile([128, NIT * D], fp32, tag="c1")
        pp = fin.tile([128, NIT * D], fp32, tag="pp")
        c2 = fin.tile([128, NIT * D], fp32, tag="c2")
        cc = fin.tile([128, NIT * D], fp32, tag="cc")
        dd1 = fin.tile([128, NIT * D], fp32, tag="dd1")
        dd = fin.tile([128, NIT * D], fp32, tag="dd")
        invs = fin.tile([128, NIT], fp32, tag="invs")
        res = fin.tile([128, NIT * D], fp32, tag="res")

        def itsl(it):
            return slice(it * D, (it + 1) * D)

        for it in range(NIT):
            nc.vector.tensor_copy(g4sb[:, it * XW:(it + 1) * XW],
                                  pp4[it][:, 0:XW])
        for it in range(NIT):
            nc.vector.tensor_tensor(g3h[:, itsl(it)], ppx[it][:, 0:D],
                                    g4sb[:, it * XW:it * XW + D], op=SUB)
            # P = (hi * r4) + G3h   (AP-scalar ops are DVE-only)
            nc.vector.scalar_tensor_tensor(pp[:, itsl(it)], hi_all[:, itsl(it)],
                                           g4sb[:, it * XW + 64:it * XW + 65],
                                           g3h[:, itsl(it)],
                                           op0=MUL, op1=ADD)
        for it in range(NIT):
            nc.vector.tensor_tensor(aa[:, itsl(it)], ppx[it][:, D:2 * D],
                                    g4sb[:, it * XW + D:it * XW + 2 * D],
                                    op=SUB)
            nc.vector.tensor_tensor(c1[:, itsl(it)], hi_all[:, itsl(it)],
                                    pp4[it][:, 65:65 + D], op=MUL)
            nc.vector.tensor_copy(ssb[:, it:it + 1], ppx[it][:, 64:65])
        # Pool: c2 = li' * P ; C = c1 - c2 ; D = D1 + C
        nc.gpsimd.tensor_tensor(c2[:], li_all[:], pp[:], op=MUL)
        nc.vector.reciprocal(invs[:], ssb[:])
        nc.gpsimd.tensor_tensor(cc[:], c1[:], c2[:], op=SUB)
        for it in range(NIT):
            # D1 = (P * -Ei) + A
            nc.vector.scalar_tensor_tensor(dd1[:, itsl(it)], pp[:, itsl(it)],
                                           negEi[:, it:it + 1], aa[:, itsl(it)],
                                           op0=MUL, op1=ADD)
        nc.gpsimd.tensor_tensor(dd[:], dd1[:], cc[:], op=ADD)
        for it in range(NIT):
            nc.vector.tensor_scalar_mul(res[:, itsl(it)], dd[:, itsl(it)],
                                        invs[:, it:it + 1])
        nc.sync.dma_start(out[:], res[:])

    nc.compile()
    return nc


def _get_program():
    if "nc" not in _CACHE:
        _CACHE["nc"] = _build_program()
    return _CACHE["nc"]


def make_in_maps(h, pe, E, A, Wk, bk, Wq, bq, beta):
    f = lambda x: np.ascontiguousarray(np.asarray(x, dtype=np.float32))
    h, pe, E, A = f(h), f(pe), f(E), f(A)
    Wk, bk, Wq, bq, beta = f(Wk), f(bk), f(Wq), f(bq), f(beta)

    L = np.log(h + 1e-8)                                    # [B,N,D]
    lip = beta[None, None, :] * L                           # beta*L
    in_maps = []
    ones_col = np.ones((N, 1), np.float32)
    rhs_c, q_c = {}, {}
    for b in range(B):
        EhL = E[:, None] * h[b] + lip[b] * h[b]
        ELp = E[:, None] + lip[b]
        R = np.concatenate([h[b], EhL, ones_col, ELp], axis=1)  # [N, 97]
        rhs_c[b] = np.ascontiguousarray(
            R.reshape(NJT, 128, GW).transpose(1, 0, 2).reshape(128, NJT * GW)
        ).astype(BF16)
        q_c[b] = np.ascontiguousarray((pe[b] @ Wq + bq).T)      # [D, N]
    for c in range(NCORES):
        b, r = c // 2, c % 2
        isl = slice(r * RPC, (r + 1) * RPC)
        atp = A[isl].T.reshape(NJT, 128, RPC).transpose(1, 0, 2)
        kT = ((pe[b, isl] @ Wk + bk) * ISD).T                   # [D, RPC]
        qkT = np.concatenate([q_c[b], kT], axis=1).astype(BF16)
        smalls = np.zeros((128, 8), np.float32)
        smalls[:, 0:NJT] = -5.0 * E.reshape(NJT, 128).T
        smalls[:, NJT:NJT + NIT] = -E[isl].reshape(NIT, 128).T
        hili = np.empty((128, 4 * D), np.float32)
        hili[:, 0:NIT * D] = h[b, isl].reshape(NIT, 128, D).transpose(
            1, 0, 2).reshape(128, NIT * D)
        hili[:, NIT * D:] = lip[b, isl].reshape(NIT, 128, D).transpose(
            1, 0, 2).reshape(128, NIT * D)
        in_maps.append({
            "erow": E[isl].reshape(1, RPC).copy(),
            "qkT": qkT,
            "at": np.ascontiguousarray(
                atp.reshape(128, NJT * RPC)).astype(BF16),
            "rhsp": rhs_c[b],
            "smalls": smalls,
            "hili": hili,
        })
    return in_maps


def gather(results):
    out = np.empty((B, N, D), np.float32)
    for c in range(NCORES):
        b, r = c // 2, c % 2
        o = results[c]["out"].reshape(128, NIT, D).transpose(1, 0, 2)
        out[b, r * RPC:(r + 1) * RPC] = o.reshape(RPC, D)
    return out


def _axon_reset():
    try:
        import ctypes
        import jax
        lib = ctypes.CDLL("/opt/axon/libaxon_pjrt.so")
        lib.axon_reset.restype = ctypes.c_int64
        jax.devices()
        lib.axon_reset()
    except Exception:
        pass


def kernel(t=None, h=None, pe=None, E=None, A=None, Wk=None, bk=None,
           Wq=None, bq=None, beta=None, **_unused):
    from concourse.bass_utils import run_bass_kernel_spmd
    nc = _get_program()
    in_maps = make_in_maps(h, pe, E, A, Wk, bk, Wq, bq, beta)
    try:
        res = run_bass_kernel_spmd(nc, in_maps, list(range(NCORES)))
    except Exception:
        # a previously wedged NeuronCore shows up as an opaque runtime
        # error on the first execute — reset the device once and retry
        _axon_reset()
        import time as _time
        _time.sleep(2)
        res = run_bass_kernel_spmd(nc, in_maps, list(range(NCORES)))
    return gather(res.results)


# revision 18
# speedup vs baseline: 1.2135x; 1.1391x over previous
"""Fused graph Fokker-Planck ODE function kernel for Trainium2 (8 NeuronCores).

Sharding: data-parallel over batch B=4 x row-halves (i in [0,256) / [256,512))
-> 8 shards.  Each core computes dh_dt for one (batch, i-half) pair.

Math (per batch; [i,j] matrices kept transposed as [j,i] on chip so the
j-contraction matmuls need no transposes):
    S      = A * (K' @ Q^T)            K' = (pe Wk + bk)/sqrt(D) (host-folded)
    X      = exp(S)                    (unnormalized softmax)
    rd     = sigmoid(10*(E_i - E_j)) = 0.5*tanh(5*(E_i - E_j)) + 0.5
    M4     = X * rd
Sign-merged accumulation (single PSUM chain per i-tile):
    acc    = X^T @ [h | EhL | 1]  +  M4^T @ [-h | -EhL | 0 | 1 | EL']
           = [G3h | A | s | r4 | G4EL']
    (EhL = E*h + beta*L*h, EL' = E + beta*L, L = log(h+1e-8) from host)
Finals:
    P   = G3h + r4*h_i
    dh  = (A - EL'_i * P + h_i * G4EL') / s
bf16/fp8 on the N^2 device path (fp32 PSUM accum); the O(N*PE*D) Q/K
projections and O(N*D) rhs packing run on host.  Measured rel err ~8e-4.
"""

import math
import sys

import numpy as np

for _p in ("/opt/trn_rl_repo",):
    if _p not in sys.path:
        sys.path.insert(0, _p)

import ml_dtypes

B, N, D, PED = 4, 512, 32, 16
NCORES = 8
RPC = N // 2            # i-rows per core
NJT = N // 128          # j tiles of 128
NIT = RPC // 128        # i tiles of 128
XW = 65                 # X-matmul rhs cols  [h|EhL|1]
MW = 98                 # M4-matmul rhs cols [-h|-EhL|0|1|EL']
TW = XW + MW            # packed rhs cols per j-tile
ISD = 1.0 / math.sqrt(D)
BF16 = ml_dtypes.bfloat16
FP8 = ml_dtypes.float8_e4m3

_CACHE = {}


def _patch_act_tables():
    """Make exp_and_others (exp + tanh + identity) the only ACT table set
    containing our functions so bacc emits exactly one ACT_TABLE_LOAD."""
    import concourse.bacc as bacc_mod
    if getattr(bacc_mod, "_act_tables_patched", False):
        return
    orig = bacc_mod.get_activation_tables

    def filtered(arch):
        t = orig(arch)
        target = t.get("exp_and_others")
        if not target:
            return t
        return {k: (v if k == "exp_and_others" else (v - target))
                for k, v in t.items()}

    bacc_mod.get_activation_tables = filtered
    bacc_mod._act_tables_patched = True


def _build_program():
    import concourse.bacc as bacc
    import concourse.tile as tile
    from concourse import mybir
    from contextlib import ExitStack

    _patch_act_tables()

    fp32 = mybir.dt.float32
    f32r = mybir.dt.float32r
    bf16 = mybir.dt.bfloat16
    fp8 = mybir.dt.float8e4
    AF = mybir.ActivationFunctionType
    ADD, MUL, SUB = (mybir.AluOpType.add, mybir.AluOpType.mult,
                     mybir.AluOpType.subtract)

    nc = bacc.Bacc("TRN2", target_bir_lowering=False, debug=False,
                   num_devices=NCORES)

    def din(name, shape, dt=fp32):
        return nc.dram_tensor(name, shape, dt, kind="ExternalInput").ap()

    erow = din("erow", [1, RPC], f32r)          # E_i row (f32r for rank-1 MM)
    qkT = din("qkT", [D, N + RPC], bf16)        # [Q^T | K'^T], host-projected
    at01 = din("at01", [128, 2 * RPC], fp8)     # A[isl].T j-tiles 0-1
    at23 = din("at23", [128, 2 * RPC], fp8)     # A[isl].T j-tiles 2-3
    rhA = din("rhA", [128, 2 * TW], bf16)       # packed rhs j-tiles 0-1
    rhB = din("rhB", [128, 2 * TW], bf16)       # packed rhs j-tiles 2-3
    smalls = din("smalls", [128, 8])            # [-5Ej(4) | pad]
    hili = din("hili", [128, 4 * D])            # [hi (NIT D) | EL'_i]
    out = nc.dram_tensor("out", [128, NIT * D], fp32,
                         kind="ExternalOutput").ap()

    with tile.TileContext(nc) as tc, ExitStack() as ctx:
        cst = ctx.enter_context(tc.tile_pool(name="cst", bufs=1))
        fin = ctx.enter_context(tc.tile_pool(name="fin", bufs=1))
        pp1 = ctx.enter_context(tc.tile_pool(name="pp1", bufs=1, space="PSUM"))
        pps = ctx.enter_context(tc.tile_pool(name="pps", bufs=4, space="PSUM"))

        # ---------------- input DMAs; three queues, critical first ---------
        erow_sb = cst.tile([1, RPC], f32r, tag="erow")
        nc.sync.dma_start(erow_sb[:], erow[:])
        qkT_sb = cst.tile([D, N + RPC], bf16, tag="qkT")
        nc.sync.dma_start(qkT_sb[:], qkT[:])
        at_sb = cst.tile([128, NJT * RPC], fp8, tag="at_sb")
        nc.sync.dma_start(at_sb[:, 0:2 * RPC], at01[:])
        rhsp_sb = cst.tile([128, NJT * TW], bf16, tag="rhsp")
        nc.sync.dma_start(rhsp_sb[:, 0:2 * TW], rhA[:])

        smalls_sb = cst.tile([128, 8], fp32, tag="smalls")
        nc.gpsimd.dma_start(smalls_sb[:], smalls[:])
        nc.gpsimd.dma_start(at_sb[:, 2 * RPC:4 * RPC], at23[:])
        nc.gpsimd.dma_start(rhsp_sb[:, 2 * TW:4 * TW], rhB[:])

        # V: constants (warm-act + eib deps)
        zero1 = cst.tile([128, 1], fp32, tag="zero1")
        nc.vector.memset(zero1[:], 0.0)
        ones1 = cst.tile([1, 128], f32r, tag="ones1")
        nc.vector.memset(ones1.bitcast(fp32)[:], 1.0)

        # hoist the single ACT_TABLE_LOAD off the critical path, then the
        # finals-only hili load on the scalar queue
        warm = cst.tile([128, 1], fp32, tag="warm")
        nc.scalar.activation(warm[:], zero1[:], AF.Exp, bias=zero1[:])
        hili_sb = cst.tile([128, 4 * D], fp32, tag="hili")
        nc.scalar.dma_start(hili_sb[:], hili[:])

        m5ej = smalls_sb[:, 0:NJT]            # -5*E_j tiles
        qT = qkT_sb[:, 0:N]
        kT = qkT_sb[:, N:N + RPC]

        # accps allocated first => PSUM banks 0-1 (one aligned pair);
        # its single start=True clear covers both accumulation chains.
        accps = pp1.tile([128, 512], fp32, tag="accps")

        # ---------------- E_i broadcast (rank-1 f32r matmul) ---------------
        ek = pp1.tile([128, RPC], fp32, tag="ek")
        nc.tensor.matmul(ek[:], ones1[:], erow_sb[:], start=True, stop=True)
        eibps = ek[:]

        # ---------------- scores (PE), tanh (S), rd (Pool) -----------------
        salls = []
        for t in range(NJT):
            sps = pps.tile([128, RPC], fp32, tag="sall")
            nc.tensor.matmul(sps[:], qT[:, t * 128:(t + 1) * 128], kT,
                             start=True, stop=True)
            salls.append(sps)

        tanh_sb = cst.tile([128, NJT * RPC], fp32, tag="tanh")
        rd_sb = cst.tile([128, NJT * RPC], bf16, tag="rd")
        msk_sb = cst.tile([128, NJT * RPC], fp32, tag="msk")
        X_sb = cst.tile([128, NJT * RPC], bf16, tag="X")
        M4_sb = cst.tile([128, NJT * RPC], bf16, tag="M4")

        def sl(t):
            return slice(t * RPC, (t + 1) * RPC)

        # scalar queue order: T0 T1 X0 T2 X1 T3 X2 X3; Pool rd_t follows
        # its tanh_t in emission order so region deps are tracked.
        for t in range(2):
            nc.scalar.activation(tanh_sb[:, sl(t)], eibps, AF.Tanh,
                                 bias=m5ej[:, t:t + 1], scale=5.0)
            nc.gpsimd.tensor_scalar(rd_sb[:, sl(t)], tanh_sb[:, sl(t)],
                                    0.5, 0.5, op0=MUL, op1=ADD)
        for t in range(NJT):
            nc.vector.tensor_tensor(msk_sb[:, sl(t)], at_sb[:, sl(t)],
                                    salls[t][:], op=MUL)
        for t in range(NJT):
            nc.scalar.activation(X_sb[:, sl(t)], msk_sb[:, sl(t)], AF.Exp,
                                 bias=zero1[:])
            if t < 2:
                nc.scalar.activation(tanh_sb[:, sl(t + 2)], eibps, AF.Tanh,
                                     bias=m5ej[:, t + 2:t + 3], scale=5.0)
                nc.gpsimd.tensor_scalar(rd_sb[:, sl(t + 2)],
                                        tanh_sb[:, sl(t + 2)],
                                        0.5, 0.5, op0=MUL, op1=ADD)
        for t in range(NJT):
            nc.vector.tensor_tensor(M4_sb[:, sl(t)], X_sb[:, sl(t)],
                                    rd_sb[:, sl(t)], op=MUL)

        # ---------------- sign-merged accumulation matmuls -----------------
        # accps[:, it*256 + c]: c in [0:32) G3h, [32:64) A, 64 s, 65 r4,
        # [66:98) G4EL'.  A start=True clears has_written for (at least)
        # the whole bank pair, so exactly ONE start heads all four chains;
        # later matmuls fresh-write where bits are clear and add elsewhere.
        first = True
        for t in range(NJT):
            sp = (t == NJT - 1)
            r1 = rhsp_sb[:, t * TW:t * TW + XW]
            r2 = rhsp_sb[:, t * TW + XW:t * TW + TW]
            for it in range(NIT):
                xsl = X_sb[:, t * RPC + it * 128:t * RPC + (it + 1) * 128]
                msl = M4_sb[:, t * RPC + it * 128:t * RPC + (it + 1) * 128]
                nc.tensor.matmul(accps[:, it * 256:it * 256 + XW], xsl, r1,
                                 start=first, stop=sp)
                first = False
                nc.tensor.matmul(accps[:, it * 256:it * 256 + MW], msl, r2,
                                 start=False, stop=sp)

        # ---------------- finals (batched over both i-tiles) ---------------
        accv = accps.rearrange("p (t c) -> p t c", c=256)
        hiv = hili_sb[:, 0:NIT * D].rearrange("p (t d) -> p t d", d=D)
        eliv = hili_sb[:, NIT * D:2 * NIT * D].rearrange(
            "p (t d) -> p t d", d=D)

        srt = fin.tile([128, NIT, 2], fp32, tag="srt")
        vv = fin.tile([128, NIT, D], fp32, tag="vv")
        pp = fin.tile([128, NIT, D], fp32, tag="pp")
        t2 = fin.tile([128, NIT, D], fp32, tag="t2")
        qq = fin.tile([128, NIT, D], fp32, tag="qq")
        uu = fin.tile([128, NIT, D], fp32, tag="uu")
        dd = fin.tile([128, NIT, D], fp32, tag="dd")
        invt = fin.tile([128, NIT], fp32, tag="invt")
        res = fin.tile([128, NIT, D], fp32, tag="res")
        iv = invt.rearrange("p (t o) -> p t o", o=1)

        nc.vector.tensor_copy(srt[:], accv[:, :, 64:66])
        nc.vector.tensor_tensor(vv[:], hiv[:],
                                srt[:, :, 1:2].to_broadcast((128, NIT, D)),
                                op=MUL)
        nc.vector.reciprocal(iv[:], srt[:, :, 0:1])
        nc.vector.tensor_tensor(pp[:], vv[:], accv[:, :, 0:D], op=ADD)
        nc.gpsimd.tensor_tensor(t2[:], eliv[:], pp[:], op=MUL)
        nc.vector.tensor_tensor(qq[:], hiv[:], accv[:, :, 66:66 + D], op=MUL)
        nc.vector.tensor_tensor(uu[:], accv[:, :, D:2 * D], t2[:], op=SUB)
        nc.gpsimd.tensor_tensor(dd[:], uu[:], qq[:], op=ADD)
        nc.vector.tensor_tensor(res[:], dd[:],
                                iv.to_broadcast((128, NIT, D)), op=MUL)
        nc.sync.dma_start(out[:], res.rearrange("p t d -> p (t d)"))

    nc.compile()
    return nc


def _get_program():
    if "nc" not in _CACHE:
        _CACHE["nc"] = _build_program()
    return _CACHE["nc"]


def make_in_maps(h, pe, E, A, Wk, bk, Wq, bq, beta):
    f = lambda x: np.ascontiguousarray(np.asarray(x, dtype=np.float32))
    h, pe, E, A = f(h), f(pe), f(E), f(A)
    Wk, bk, Wq, bq, beta = f(Wk), f(bk), f(Wq), f(bq), f(beta)

    L = np.log(h + 1e-8)                                    # [B,N,D]
    lip = beta[None, None, :] * L                           # beta*L
    in_maps = []
    ones_col = np.ones((N, 1), np.float32)
    zeros_col = np.zeros((N, 1), np.float32)
    rhs_c, q_c = {}, {}
    for b in range(B):
        EhL = E[:, None] * h[b] + lip[b] * h[b]
        ELp = E[:, None] + lip[b]
        # [h | EhL | 1 || -h | -EhL | 0 | 1 | EL']  -> [N, 163]
        R = np.concatenate([h[b], EhL, ones_col,
                            -h[b], -EhL, zeros_col, ones_col, ELp], axis=1)
        rhs_c[b] = np.ascontiguousarray(
            R.reshape(NJT, 128, TW).transpose(1, 0, 2).reshape(128, NJT * TW)
        ).astype(BF16)
        q_c[b] = np.ascontiguousarray((pe[b] @ Wq + bq).T)      # [D, N]
    for c in range(NCORES):
        b, r = c // 2, c % 2
        isl = slice(r * RPC, (r + 1) * RPC)
        atp = A[isl].T.reshape(NJT, 128, RPC).transpose(1, 0, 2).reshape(
            128, NJT * RPC).astype(FP8)
        kT = ((pe[b, isl] @ Wk + bk) * ISD).T                   # [D, RPC]
        qkT = np.concatenate([q_c[b], kT], axis=1).astype(BF16)
        smalls = np.zeros((128, 8), np.float32)
        smalls[:, 0:NJT] = -5.0 * E.reshape(NJT, 128).T
        hili = np.empty((128, 4 * D), np.float32)
        hili[:, 0:NIT * D] = h[b, isl].reshape(NIT, 128, D).transpose(
            1, 0, 2).reshape(128, NIT * D)
        ELp_b = E[isl, None] + lip[b, isl]
        hili[:, NIT * D:] = ELp_b.reshape(NIT, 128, D).transpose(
            1, 0, 2).reshape(128, NIT * D)
        in_maps.append({
            "erow": E[isl].reshape(1, RPC).copy(),
            "qkT": qkT,
            "at01": np.ascontiguousarray(atp[:, 0:2 * RPC]),
            "at23": np.ascontiguousarray(atp[:, 2 * RPC:4 * RPC]),
            "rhA": np.ascontiguousarray(rhs_c[b][:, 0:2 * TW]),
            "rhB": np.ascontiguousarray(rhs_c[b][:, 2 * TW:4 * TW]),
            "smalls": smalls,
            "hili": hili,
        })
    return in_maps


def gather(results):
    out = np.empty((B, N, D), np.float32)
    for c in range(NCORES):
        b, r = c // 2, c % 2
        o = results[c]["out"].reshape(128, NIT, D).transpose(1, 0, 2)
        out[b, r * RPC:(r + 1) * RPC] = o.reshape(RPC, D)
    return out


def _axon_reset():
    try:
        import ctypes
        import jax
        lib = ctypes.CDLL("/opt/axon/libaxon_pjrt.so")
        lib.axon_reset.restype = ctypes.c_int64
        jax.devices()
        lib.axon_reset()
    except Exception:
        pass


def kernel(t=None, h=None, pe=None, E=None, A=None, Wk=None, bk=None,
           Wq=None, bq=None, beta=None, **_unused):
    from concourse.bass_utils import run_bass_kernel_spmd
    nc = _get_program()
    in_maps = make_in_maps(h, pe, E, A, Wk, bk, Wq, bq, beta)
    try:
        res = run_bass_kernel_spmd(nc, in_maps, list(range(NCORES)))
    except Exception:
        # a previously wedged NeuronCore shows up as an opaque runtime
        # error on the first execute — reset the device once and retry
        _axon_reset()
        import time as _time
        _time.sleep(2)
        res = run_bass_kernel_spmd(nc, in_maps, list(range(NCORES)))
    return gather(res.results)


# revision 19
# speedup vs baseline: 1.2325x; 1.0157x over previous
"""Fused graph Fokker-Planck ODE function kernel for Trainium2 (8 NeuronCores).

Sharding: data-parallel over batch B=4 x row-halves (i in [0,256) / [256,512))
-> 8 shards.  Each core computes dh_dt for one (batch, i-half) pair.

Math (per batch; [i,j] matrices kept transposed as [j,i] on chip so the
j-contraction matmuls need no transposes):
    S      = A * (K' @ Q^T)            K' = (pe Wk + bk)/sqrt(D) (host-folded)
    X      = exp(S)                    (unnormalized softmax)
    rd'    = -sigmoid(10*(E_i - E_j)) = -0.5*tanh(5*(E_i - E_j)) - 0.5
    M4n    = X * rd'                   (negated so X and M4n share one rhs)
Single-PSUM-chain accumulation over a shared packed rhs [h|EhL|1|EL'|1]:
    acc    = X^T @ rhs[:, 0:65]  +  M4n^T @ rhs[:, 0:98]
           = [G3h | A | r3 | -G4EL' | -r4]
    (EhL = E*h + beta*L*h, EL' = E + beta*L, L = log(h+1e-8) from host)
Finals:
    s = r3 + r4;  P = G3h + r4*h_i
    dh = (A - EL'_i * P + h_i * G4EL') / s
bf16/fp8 on the N^2 device path (fp32 PSUM accum); the O(N*PE*D) Q/K
projections and O(N*D) rhs packing run on host.  Measured rel err ~8e-4.
"""

import math
import sys

import numpy as np

for _p in ("/opt/trn_rl_repo",):
    if _p not in sys.path:
        sys.path.insert(0, _p)

import ml_dtypes

B, N, D, PED = 4, 512, 32, 16
NCORES = 8
RPC = N // 2            # i-rows per core
NJT = N // 128          # j tiles of 128
NIT = RPC // 128        # i tiles of 128
XW = 65                 # X-matmul rhs cols  [h|EhL|1]
MW = 98                 # M4n-matmul rhs cols [h|EhL|1|EL'|1]
TW = MW                 # packed rhs cols per j-tile (fully shared)
ISD = 1.0 / math.sqrt(D)
BF16 = ml_dtypes.bfloat16
FP8 = ml_dtypes.float8_e4m3

_CACHE = {}


def _patch_act_tables():
    """Make exp_and_others (exp + tanh + identity) the only ACT table set
    containing our functions so bacc emits exactly one ACT_TABLE_LOAD."""
    import concourse.bacc as bacc_mod
    if getattr(bacc_mod, "_act_tables_patched", False):
        return
    orig = bacc_mod.get_activation_tables

    def filtered(arch):
        t = orig(arch)
        target = t.get("exp_and_others")
        if not target:
            return t
        return {k: (v if k == "exp_and_others" else (v - target))
                for k, v in t.items()}

    bacc_mod.get_activation_tables = filtered
    bacc_mod._act_tables_patched = True


def _build_program():
    import concourse.bacc as bacc
    import concourse.tile as tile
    from concourse import mybir
    from contextlib import ExitStack

    _patch_act_tables()

    fp32 = mybir.dt.float32
    f32r = mybir.dt.float32r
    bf16 = mybir.dt.bfloat16
    fp8 = mybir.dt.float8e4
    AF = mybir.ActivationFunctionType
    ADD, MUL, SUB = (mybir.AluOpType.add, mybir.AluOpType.mult,
                     mybir.AluOpType.subtract)

    nc = bacc.Bacc("TRN2", target_bir_lowering=False, debug=False,
                   num_devices=NCORES)

    def din(name, shape, dt=fp32):
        return nc.dram_tensor(name, shape, dt, kind="ExternalInput").ap()

    smalls = din("smalls", [128, 8])            # [-5Ej(4) | pad]
    erow = din("erow", [1, RPC], f32r)          # E_i row (f32r for rank-1 MM)
    qkT = din("qkT", [D, N + RPC], bf16)        # [Q^T | K'^T], host-projected
    rhA = din("rhA", [128, 2 * TW], bf16)       # packed rhs j-tiles 0-1
    at01 = din("at01", [128, 2 * RPC], fp8)     # A[isl].T j-tiles 0-1
    at23 = din("at23", [128, 2 * RPC], fp8)     # A[isl].T j-tiles 2-3
    rhB = din("rhB", [128, 2 * TW], bf16)       # packed rhs j-tiles 2-3
    hili = din("hili", [128, 4 * D])            # [hi (NIT D) | EL'_i]
    out = nc.dram_tensor("out", [128, NIT * D], fp32,
                         kind="ExternalOutput").ap()

    with tile.TileContext(nc) as tc, ExitStack() as ctx:
        cst = ctx.enter_context(tc.tile_pool(name="cst", bufs=1))
        fin = ctx.enter_context(tc.tile_pool(name="fin", bufs=1))
        pp1 = ctx.enter_context(tc.tile_pool(name="pp1", bufs=1, space="PSUM"))

        # ---------------- input DMAs; sync takes the early-needed ----------
        smalls_sb = cst.tile([128, 8], fp32, tag="smalls")
        nc.sync.dma_start(smalls_sb[:], smalls[:])
        erow_sb = cst.tile([1, RPC], f32r, tag="erow")
        nc.sync.dma_start(erow_sb[:], erow[:])
        qkT_sb = cst.tile([D, N + RPC], bf16, tag="qkT")
        nc.sync.dma_start(qkT_sb[:], qkT[:])
        rhsp_sb = cst.tile([128, NJT * TW], bf16, tag="rhsp")
        nc.sync.dma_start(rhsp_sb[:, 0:2 * TW], rhA[:])

        # V: constants (warm-act + eib deps)
        zero1 = cst.tile([128, 1], fp32, tag="zero1")
        nc.vector.memset(zero1[:], 0.0)
        ones1 = cst.tile([1, 128], f32r, tag="ones1")
        nc.vector.memset(ones1.bitcast(fp32)[:], 1.0)

        # scalar queue: ACT table + warm first, then the mask halves
        warm = cst.tile([128, 1], fp32, tag="warm")
        nc.scalar.activation(warm[:], zero1[:], AF.Exp, bias=zero1[:])
        at_sb = cst.tile([128, NJT * RPC], fp8, tag="at_sb")
        nc.scalar.dma_start(at_sb[:, 0:2 * RPC], at01[:])
        nc.scalar.dma_start(at_sb[:, 2 * RPC:4 * RPC], at23[:])

        # gpsimd queue (slowest DMA path): late-needed tensors
        nc.gpsimd.dma_start(rhsp_sb[:, 2 * TW:4 * TW], rhB[:])
        hili_sb = cst.tile([128, 4 * D], fp32, tag="hili")
        nc.gpsimd.dma_start(hili_sb[:], hili[:])

        m5ej = smalls_sb[:, 0:NJT]            # -5*E_j tiles
        qT = qkT_sb[:, 0:N]
        kT = qkT_sb[:, N:N + RPC]

        # accps allocated first => PSUM banks 0-1 (one aligned pair);
        # its single start=True clear covers both accumulation chains.
        accps = pp1.tile([128, 512], fp32, tag="accps")

        # ---------------- E_i broadcast (rank-1 f32r matmul) ---------------
        ek = pp1.tile([128, RPC], fp32, tag="ek")
        nc.tensor.matmul(ek[:], ones1[:], erow_sb[:], start=True, stop=True)
        eibps = ek[:]

        # ---------------- scores into one 4-bank PSUM tile -----------------
        sall = pp1.tile([128, NJT * RPC], fp32, tag="sall")
        for t in range(NJT):
            nc.tensor.matmul(sall[:, t * RPC:(t + 1) * RPC],
                             qT[:, t * 128:(t + 1) * 128], kT,
                             start=True, stop=True)

        tanh_sb = cst.tile([128, NJT * RPC], fp32, tag="tanh")
        rd_sb = cst.tile([128, NJT * RPC], bf16, tag="rd")
        msk_sb = cst.tile([128, NJT * RPC], fp32, tag="msk")
        X_sb = cst.tile([128, NJT * RPC], bf16, tag="X")
        M4_sb = cst.tile([128, NJT * RPC], bf16, tag="M4")

        def sl(t):
            return slice(t * RPC, (t + 1) * RPC)

        def dl(p):
            return slice(p * 2 * RPC, (p + 1) * 2 * RPC)

        # S queue: T0 T1 X01 T2 T3 X23; Pool rd' halves follow their tanhs
        for t in range(2):
            nc.scalar.activation(tanh_sb[:, sl(t)], eibps, AF.Tanh,
                                 bias=m5ej[:, t:t + 1], scale=5.0)
        nc.gpsimd.tensor_scalar(rd_sb[:, dl(0)], tanh_sb[:, dl(0)],
                                -0.5, -0.5, op0=MUL, op1=ADD)
        nc.vector.tensor_tensor(msk_sb[:, dl(0)], at_sb[:, dl(0)],
                                sall[:, dl(0)], op=MUL)
        nc.scalar.activation(X_sb[:, dl(0)], msk_sb[:, dl(0)], AF.Exp,
                             bias=zero1[:])
        for t in range(2, NJT):
            nc.scalar.activation(tanh_sb[:, sl(t)], eibps, AF.Tanh,
                                 bias=m5ej[:, t:t + 1], scale=5.0)
        nc.gpsimd.tensor_scalar(rd_sb[:, dl(1)], tanh_sb[:, dl(1)],
                                -0.5, -0.5, op0=MUL, op1=ADD)
        nc.vector.tensor_tensor(msk_sb[:, dl(1)], at_sb[:, dl(1)],
                                sall[:, dl(1)], op=MUL)
        nc.scalar.activation(X_sb[:, dl(1)], msk_sb[:, dl(1)], AF.Exp,
                             bias=zero1[:])
        nc.vector.tensor_tensor(M4_sb[:, dl(0)], X_sb[:, dl(0)],
                                rd_sb[:, dl(0)], op=MUL)
        nc.vector.tensor_tensor(M4_sb[:, dl(1)], X_sb[:, dl(1)],
                                rd_sb[:, dl(1)], op=MUL)

        # ---------------- shared-rhs accumulation matmuls ------------------
        # accps[:, it*256 + c]: c in [0:32) G3h, [32:64) A, 64 r3,
        # [65:97) -G4EL', 97 -r4.  A start=True clears has_written for the
        # whole bank pair, so exactly ONE start heads all four chains.
        first = True
        for t in range(NJT):
            sp = (t == NJT - 1)
            rs = rhsp_sb[:, t * TW:t * TW + TW]
            for it in range(NIT):
                xsl = X_sb[:, t * RPC + it * 128:t * RPC + (it + 1) * 128]
                msl = M4_sb[:, t * RPC + it * 128:t * RPC + (it + 1) * 128]
                nc.tensor.matmul(accps[:, it * 256:it * 256 + XW], xsl,
                                 rs[:, 0:XW], start=first, stop=sp)
                first = False
                nc.tensor.matmul(accps[:, it * 256:it * 256 + MW], msl, rs,
                                 start=False, stop=sp)

        # ---------------- finals (batched over both i-tiles) ---------------
        accv = accps.rearrange("p (t c) -> p t c", c=256)
        hiv = hili_sb[:, 0:NIT * D].rearrange("p (t d) -> p t d", d=D)
        eliv = hili_sb[:, NIT * D:2 * NIT * D].rearrange(
            "p (t d) -> p t d", d=D)

        srt = fin.tile([128, NIT, 1], fp32, tag="srt")
        ss = fin.tile([128, NIT, 1], fp32, tag="ss")
        vv = fin.tile([128, NIT, D], fp32, tag="vv")
        pp = fin.tile([128, NIT, D], fp32, tag="pp")
        t2 = fin.tile([128, NIT, D], fp32, tag="t2")
        qq = fin.tile([128, NIT, D], fp32, tag="qq")
        uu = fin.tile([128, NIT, D], fp32, tag="uu")
        dd = fin.tile([128, NIT, D], fp32, tag="dd")
        invt = fin.tile([128, NIT], fp32, tag="invt")
        res = fin.tile([128, NIT, D], fp32, tag="res")
        iv = invt.rearrange("p (t o) -> p t o", o=1)

        # s = r3 - (-r4);  P = G3h - (-r4)*hi
        nc.vector.tensor_copy(srt[:], accv[:, :, 64:65])
        nc.vector.tensor_tensor(ss[:], srt[:], accv[:, :, 97:98], op=SUB)
        nc.vector.tensor_tensor(vv[:], hiv[:],
                                accv[:, :, 97:98].to_broadcast((128, NIT, D)),
                                op=MUL)
        nc.vector.reciprocal(iv[:], ss[:])
        nc.vector.tensor_tensor(pp[:], accv[:, :, 0:D], vv[:], op=SUB)
        nc.gpsimd.tensor_tensor(t2[:], eliv[:], pp[:], op=MUL)
        nc.vector.tensor_tensor(qq[:], hiv[:], accv[:, :, 65:65 + D], op=MUL)
        nc.vector.tensor_tensor(uu[:], accv[:, :, D:2 * D], t2[:], op=SUB)
        nc.gpsimd.tensor_tensor(dd[:], uu[:], qq[:], op=SUB)
        nc.vector.tensor_tensor(res[:], dd[:],
                                iv.to_broadcast((128, NIT, D)), op=MUL)
        nc.sync.dma_start(out[:], res.rearrange("p t d -> p (t d)"))

    nc.compile()
    return nc


def _get_program():
    if "nc" not in _CACHE:
        _CACHE["nc"] = _build_program()
    return _CACHE["nc"]


def make_in_maps(h, pe, E, A, Wk, bk, Wq, bq, beta):
    f = lambda x: np.ascontiguousarray(np.asarray(x, dtype=np.float32))
    h, pe, E, A = f(h), f(pe), f(E), f(A)
    Wk, bk, Wq, bq, beta = f(Wk), f(bk), f(Wq), f(bq), f(beta)

    L = np.log(h + 1e-8)                                    # [B,N,D]
    lip = beta[None, None, :] * L                           # beta*L
    in_maps = []
    ones_col = np.ones((N, 1), np.float32)
    rhs_c, q_c = {}, {}
    for b in range(B):
        EhL = E[:, None] * h[b] + lip[b] * h[b]
        ELp = E[:, None] + lip[b]
        # [h | EhL | 1 | EL' | 1]  -> [N, 98]
        R = np.concatenate([h[b], EhL, ones_col, ELp, ones_col], axis=1)
        rhs_c[b] = np.ascontiguousarray(
            R.reshape(NJT, 128, TW).transpose(1, 0, 2).reshape(128, NJT * TW)
        ).astype(BF16)
        q_c[b] = np.ascontiguousarray((pe[b] @ Wq + bq).T)      # [D, N]
    for c in range(NCORES):
        b, r = c // 2, c % 2
        isl = slice(r * RPC, (r + 1) * RPC)
        atp = A[isl].T.reshape(NJT, 128, RPC).transpose(1, 0, 2).reshape(
            128, NJT * RPC).astype(FP8)
        kT = ((pe[b, isl] @ Wk + bk) * ISD).T                   # [D, RPC]
        qkT = np.concatenate([q_c[b], kT], axis=1).astype(BF16)
        smalls = np.zeros((128, 8), np.float32)
        smalls[:, 0:NJT] = -5.0 * E.reshape(NJT, 128).T
        hili = np.empty((128, 4 * D), np.float32)
        hili[:, 0:NIT * D] = h[b, isl].reshape(NIT, 128, D).transpose(
            1, 0, 2).reshape(128, NIT * D)
        ELp_b = E[isl, None] + lip[b, isl]
        hili[:, NIT * D:] = ELp_b.reshape(NIT, 128, D).transpose(
            1, 0, 2).reshape(128, NIT * D)
        in_maps.append({
            "smalls": smalls,
            "erow": E[isl].reshape(1, RPC).copy(),
            "qkT": qkT,
            "rhA": np.ascontiguousarray(rhs_c[b][:, 0:2 * TW]),
            "at01": np.ascontiguousarray(atp[:, 0:2 * RPC]),
            "at23": np.ascontiguousarray(atp[:, 2 * RPC:4 * RPC]),
            "rhB": np.ascontiguousarray(rhs_c[b][:, 2 * TW:4 * TW]),
            "hili": hili,
        })
    return in_maps


def gather(results):
    out = np.empty((B, N, D), np.float32)
    for c in range(NCORES):
        b, r = c // 2, c % 2
        o = results[c]["out"].reshape(128, NIT, D).transpose(1, 0, 2)
        out[b, r * RPC:(r + 1) * RPC] = o.reshape(RPC, D)
    return out


def _axon_reset():
    try:
        import ctypes
        import jax
        lib = ctypes.CDLL("/opt/axon/libaxon_pjrt.so")
        lib.axon_reset.restype = ctypes.c_int64
        jax.devices()
        lib.axon_reset()
    except Exception:
        pass


def kernel(t=None, h=None, pe=None, E=None, A=None, Wk=None, bk=None,
           Wq=None, bq=None, beta=None, **_unused):
    from concourse.bass_utils import run_bass_kernel_spmd
    nc = _get_program()
    in_maps = make_in_maps(h, pe, E, A, Wk, bk, Wq, bq, beta)
    try:
        res = run_bass_kernel_spmd(nc, in_maps, list(range(NCORES)))
    except Exception:
        # a previously wedged NeuronCore shows up as an opaque runtime
        # error on the first execute — reset the device once and retry
        _axon_reset()
        import time as _time
        _time.sleep(2)
        res = run_bass_kernel_spmd(nc, in_maps, list(range(NCORES)))
    return gather(res.results)
